# revision 37
# baseline (speedup 1.0000x reference)
"""Trainium2 Bass kernel for nn_ConnectionTransformer (8 NeuronCores, SPMD).

Strategy (v2 — bf16 weight streaming)
-------------------------------------
- Phase A (embed + compress attention): batch-parallel, core c handles batch c.
- Phase B (6 bilinear message-passing steps): target-slot sharding — core c owns
  16 target slots j in [16c, 16c+16). The per-pair weights W_source/W_target are
  converted to bf16 on the host (1 GB -> 512 MB total, j-sharded 8 ways) and
  streamed from HBM once per step per core — the memory roofline.
  Source slots are processed in groups g of IL=4 so the second einsum contracts
  over a full 128-deep (il, r) partition axis in a single matmul per target
  slot.  State h is kept both as bf16 h^T (matmul operand) and as an f32
  "sparse" (slot,batch)-row layout for relu/residual/LayerNorm.  A bf16
  AllGather rebuilds the replicated h^T each step.
- Phase C (expand attention + vocab projection): batch-parallel; W_out and the
  logits are bf16 (host casts logits back to f32).
"""
import os
import sys

sys.path.insert(0, "/opt/trn_rl_repo")

import numpy as np
from concourse import bass, bacc, tile, bass_utils, mybir
from concourse import masks

B, L, D, S, R, STEPS, V = 8, 512, 256, 128, 32, 6, 32000
NC = 8
JL = S // NC          # 16 local target slots per core
IL = 4                # source slots per group
G = S // IL           # 32 source groups
VT = 500              # vocab tile width
NVT = V // VT         # 64 vocab tiles
SCALE = 1.0 / np.sqrt(D)
LN_EPS = 1e-5

F32 = mybir.dt.float32
BF16 = mybir.dt.bfloat16
BF16_NP = mybir.dt.np(mybir.dt.bfloat16)

N_STEPS = int(os.environ.get("N_STEPS", str(STEPS)))
WBUFS = int(os.environ.get("WBUFS", "3"))
CGP = int(os.environ.get("CGP", "2"))


# ---------------------------------------------------------------------------
# Device program
# ---------------------------------------------------------------------------

def build():
    nc = bacc.Bacc("TRN2", target_bir_lowering=False, debug=False, num_devices=NC)

    io = {}

    def inp(name, shape, dtype=F32):
        io[name] = nc.dram_tensor(name, shape, dtype, kind="ExternalInput").ap()

    inp("xT_in", [D, L])
    inp("maskw", [128, 4])
    for w in ("wqT", "wkslT", "wvT", "wqoT", "wkfT", "wvfT"):
        inp(w, [D, D])
    inp("hT_in", [D, S])
    inp("hn_in", [S, D])
    inp("lngb", [128, STEPS, 2 * D])
    inp("wsrc", [G // 2, 128, 2, IL, JL, 2, R], BF16)
    inp("wtgt", [G // 2, 128, 2, JL, D], BF16)
    inp("woutT", [NVT, 2, 128, VT], BF16)
    io["lg_out"] = nc.dram_tensor(
        "lg_out", [NVT, 128, 4, VT], BF16, kind="ExternalOutput"
    ).ap()

    with tile.TileContext(nc) as tc:
        _body(nc, tc, io)
    nc.compile()
    return nc


def _body(nc, tc, io):
    with tc.tile_pool(name="const", bufs=1) as const, \
         tc.tile_pool(name="state", bufs=1) as state, \
         tc.tile_pool(name="isbp", bufs=3) as isbp, \
         tc.tile_pool(name="sbp", bufs=1) as sbp, \
         tc.tile_pool(name="drp", bufs=2, space="DRAM") as drp:

        ident = const.tile([128, 128], F32)
        masks.make_identity(nc, ident[:])
        ones = const.tile([128, 1], F32)
        nc.vector.memset(ones[:], 1.0)
        eps_sb = const.tile([128, 1], F32)
        nc.vector.memset(eps_sb[:], LN_EPS)

        pid = nc.sync.partition_id()

        # tiny dummy AllGather to absorb the first-collective bringup latency
        # in parallel with phase A
        wup_in = drp.tile([32], F32, tag="wup_in")
        wup_out = drp.tile([NC, 32], F32, addr_space="Shared", tag="wup_out")
        nc.gpsimd.collective_compute(
            "AllGather", mybir.AluOpType.bypass,
            ins=[wup_in[:].opt()], outs=[wup_out[:].opt()],
            replica_groups=[list(range(NC))],
        )

        # persistent state
        # h^T for matmuls: [dp, slot, batch] bf16, one tile per 128-d half
        hT_bf = [state.tile([128, S, B], BF16, name=f"hTbf{dt}") for dt in range(2)]
        # own slots, natural layout, f32, "sparse" rows: partition 32m+b holds
        # slot jl=4q+m of batch b at free index q  (rows 8..31 of each 32-block
        # are unused garbage)
        h_upd = state.tile([128, 4, D], F32)
        qoT = [state.tile([128, L], F32, name=f"qoT{pt}") for pt in range(2)]
        lngb_sb = state.tile([128, STEPS, 2 * D], F32)
        nc.gpsimd.dma_start(lngb_sb[:], io["lngb"])

        # weight streaming pools live through phase A (prefetch) + the steps,
        # released before phase C so its pools get the SBUF space back
        with tc.tile_pool(name="wsp", bufs=WBUFS) as wsp, \
             tc.tile_pool(name="wtp", bufs=WBUFS) as wtp:
            _phase_a(nc, tc, io, ident, ones, pid, hT_bf, h_upd, qoT)
            # persistent cache for the first CGP g-pairs: loaded once (after
            # phase A frees its SBUF), reused by all steps.  Cached pairs are
            # the step head, so compute restarts right after each AllGather
            # without waiting on weight DMA.
            with tc.tile_pool(name="wcache", bufs=1) as wc, \
                 tc.tile_pool(name="ipsp", bufs=2, space="PSUM") as ipsp, \
                 tc.tile_pool(name="p2p", bufs=1, space="PSUM") as p2p:
                cws, cwt = [], []
                for k in range(CGP):
                    c_ws = wc.tile([128, 2, IL, JL, 2, R], BF16, name=f"cws{k}")
                    nc.sync.dma_start(c_ws[:], io["wsrc"][k])
                    cws.append(c_ws)
                    c_wt = wc.tile([128, 2, JL, D], BF16, name=f"cwt{k}")
                    nc.scalar.dma_start(c_wt[:], io["wtgt"][k])
                    cwt.append(c_wt)
                # streamed-weight prefetch handoff: tiles DMA'd before step
                # t's AllGather, consumed at the head of step t+1
                pending = {}
                _prefetch(nc, io, wsp, wtp, pending, 0)
                for t in range(N_STEPS):
                    _step(nc, tc, t, io, hT_bf, h_upd, lngb_sb, ident, eps_sb,
                          wsp, wtp, isbp, sbp, ipsp, p2p, drp, cws, cwt,
                          pending, prefetch_next=(t + 1 < N_STEPS))
        _phase_c(nc, tc, io, ident, pid, hT_bf, qoT)


def _phase_a(nc, tc, io, ident, ones, pid, hT_bf, h_upd, qoT):
    with tc.tile_pool(name="pa_sb", bufs=1) as pa, \
         tc.tile_pool(name="pa_ps", bufs=3, space="PSUM") as pps, \
         tc.tile_pool(name="pa_tp", bufs=2, space="PSUM") as tps, \
         tc.tile_pool(name="pa_acc", bufs=1, space="PSUM") as aps, \
         tc.tile_pool(name="dram_a", bufs=1, space="DRAM") as dra:

        mask_sb = pa.tile([128, 4], F32)
        nc.gpsimd.dma_start(mask_sb[:], io["maskw"])

        # X^T tiles [d128, t512] (host-gathered embeddings, transposed)
        xT = [pa.tile([128, L], F32, name=f"xT{ct}") for ct in range(2)]
        for ct in range(2):
            nc.gpsimd.dma_start(xT[ct][:], io["xT_in"][128 * ct : 128 * (ct + 1), :])

        # weight tiles [d128, 256] (contraction on partitions)
        def load_w(name):
            ts = [pa.tile([128, D], F32, name=f"{name}_{ct}") for ct in range(2)]
            for ct in range(2):
                nc.gpsimd.dma_start(ts[ct][:], io[name][128 * ct : 128 * (ct + 1), :])
            return ts

        wq_sb = load_w("wqT")
        wv_sb = load_w("wvT")
        wksl_sb = load_w("wkslT")
        wqo_sb = load_w("wqoT")
        hTt = [pa.tile([128, S], F32, name=f"hTt{ct}") for ct in range(2)]
        for ct in range(2):
            nc.gpsimd.dma_start(hTt[ct][:], io["hT_in"][128 * ct : 128 * (ct + 1), :])
        hn_sb = pa.tile([S, D], F32)
        nc.gpsimd.dma_start(hn_sb[:], io["hn_in"])

        # Q_in^T and Q_out^T : [d'128 x 2, t512]
        qT = [pa.tile([128, L], F32, name=f"qT{pt}") for pt in range(2)]
        for pt in range(2):
            for dst, wsb in ((qT, wq_sb), (qoT, wqo_sb)):
                ps = pps.tile([128, L], F32, tag="ps")
                for ct in range(2):
                    nc.tensor.matmul(
                        ps[:], wsb[ct][:, 128 * pt : 128 * (pt + 1)], xT[ct][:],
                        start=(ct == 0), stop=(ct == 1),
                    )
                nc.vector.tensor_copy(dst[pt][:], ps[:])

        # V_in natural [t128 x 4, d256]
        vn = pa.tile([128, 4, D], F32)
        for tt in range(4):
            ps = pps.tile([128, L], F32, tag="ps")
            for ct in range(2):
                nc.tensor.matmul(
                    ps[:, 0:D], xT[ct][:, 128 * tt : 128 * (tt + 1)], wv_sb[ct][:],
                    start=(ct == 0), stop=(ct == 1),
                )
            nc.vector.tensor_copy(vn[:, tt, :], ps[:, 0:D])

        # K_slots^T [d'128 x 2, s128]
        kslT = [pa.tile([128, S], F32, name=f"kslT{pt}") for pt in range(2)]
        for pt in range(2):
            ps = pps.tile([128, L], F32, tag="ps")
            for ct in range(2):
                nc.tensor.matmul(
                    ps[:, 0:S], wksl_sb[ct][:, 128 * pt : 128 * (pt + 1)], hTt[ct][:],
                    start=(ct == 0), stop=(ct == 1),
                )
            nc.vector.tensor_copy(kslT[pt][:], ps[:, 0:S])

        # attention scores + masked softmax
        a_sb = pa.tile([128, 4, S], F32)
        for tt in range(4):
            sc = pps.tile([128, L], F32, tag="ps")
            for pt in range(2):
                nc.tensor.matmul(
                    sc[:, 0:S], qT[pt][:, 128 * tt : 128 * (tt + 1)], kslT[pt][:],
                    start=(pt == 0), stop=(pt == 1),
                )
            rowmax = pa.tile([128, 1], F32, tag="rmax")
            nc.vector.tensor_reduce(
                rowmax[:], sc[:, 0:S], axis=mybir.AxisListType.X,
                op=mybir.AluOpType.max,
            )
            nb = pa.tile([128, 1], F32, tag="nb")
            nc.vector.tensor_scalar_mul(nb[:], rowmax[:], -SCALE)
            sumexp = pa.tile([128, 1], F32, tag="sexp")
            nc.scalar.activation(
                a_sb[:, tt, :], sc[:, 0:S], mybir.ActivationFunctionType.Exp,
                bias=nb[:], scale=SCALE, accum_out=sumexp[:],
            )
            rs = pa.tile([128, 1], F32, tag="rs")
            nc.vector.reciprocal(rs[:], sumexp[:])
            rm = pa.tile([128, 1], F32, tag="rmk")
            nc.vector.tensor_tensor(
                rm[:], rs[:], mask_sb[:, tt : tt + 1], op=mybir.AluOpType.mult
            )
            nc.vector.tensor_scalar_mul(a_sb[:, tt, :], a_sb[:, tt, :], rm[:])

        # column sums and IR = A^T @ V
        cs = aps.tile([128, 1], F32, tag="cs")
        for tt in range(4):
            nc.tensor.matmul(
                cs[:], a_sb[:, tt, :], ones[:, 0:1], start=(tt == 0), stop=(tt == 3)
            )
        ir = aps.tile([128, D], F32, tag="ir")
        for tt in range(4):
            nc.tensor.matmul(
                ir[:], a_sb[:, tt, :], vn[:, tt, :], start=(tt == 0), stop=(tt == 3)
            )
        cssb = pa.tile([128, 1], F32)
        nc.vector.tensor_scalar_add(cssb[:], cs[:], 1e-8)
        rcs = pa.tile([128, 1], F32)
        nc.vector.reciprocal(rcs[:], cssb[:])
        h0 = pa.tile([S, D], F32)
        nc.vector.scalar_tensor_tensor(
            h0[:], ir[:], rcs[:], hn_sb[:],
            op0=mybir.AluOpType.mult, op1=mybir.AluOpType.add,
        )

        # h0 natural bounce -> AllGather over batches (f32, 128 KB per core)
        agin0 = dra.tile([S * D], F32)
        nc.gpsimd.dma_start(
            agin0[:].rearrange("(p f) -> p f", p=128), h0[:]
        )
        agout0 = dra.tile([NC, S * D], F32, addr_space="Shared")
        nc.gpsimd.collective_compute(
            "AllGather", mybir.AluOpType.bypass,
            ins=[agin0[:].opt()], outs=[agout0[:].opt()],
            replica_groups=[list(range(NC))],
        )
        # readback per batch (contiguous), transpose on the PE, cast into the
        # bf16 h^T state
        for b in range(B):
            hnat = pa.tile([S, D], F32, tag="hnat", name="hnat", bufs=2)
            nc.gpsimd.dma_start(
                hnat[:], agout0[b].rearrange("(s d) -> s d", d=D)
            )
            for dt in range(2):
                p3 = tps.tile([128, 128], F32, tag="tp")
                nc.tensor.transpose(
                    p3[:], hnat[:, 128 * dt : 128 * (dt + 1)], ident[:]
                )
                nc.vector.tensor_copy(hT_bf[dt][:, :, b : b + 1], p3[:])
        # own 16 slots (all batches) into the sparse f32 residual layout:
        # h_upd[32m+b, q, d] = h0_b[slot 16c+4q+m, d]
        nat = agout0[:][:, bass.ds(pid * (JL * D), JL * D)]
        natr = nat.rearrange("b (q m d) -> m b q d", q=4, m=4, d=D)
        for m in range(4):
            nc.sync.dma_start(h_upd[32 * m : 32 * m + B, :, :], natr[m])


def _prefetch(nc, io, wsp, wtp, pending, _unused):
    """DMA the first WBUFS streamed g-pairs of the next step into fresh pool
    buffers; the next _step call picks them up from `pending`."""
    for gp in range(CGP, min(CGP + WBUFS, G // 2)):
        ws = wsp.tile([128, 2, IL, JL, 2, R], BF16, tag="ws", name="ws_pf")
        nc.sync.dma_start(ws[:], io["wsrc"][gp])
        wt = wtp.tile([128, 2, JL, D], BF16, tag="wt", name="wt_pf")
        nc.scalar.dma_start(wt[:], io["wtgt"][gp])
        pending[gp] = (ws, wt)


def _step(nc, tc, t, io, hT_bf, h_upd, lngb_sb, ident, eps_sb,
          wsp, wtp, isbp, sbp, ipsp, p2p, drp, cws, cwt, pending,
          prefetch_next):
    """One message-passing step."""
    wsrc, wtgt = io["wsrc"], io["wtgt"]
    # influence accumulators: partition 32m+b of tile q holds slot jl=4q+m.
    # One full (padded) PSUM bank per q so each (q, m) accumulation group owns
    # its whole 2KB zero region.
    p2 = [
        p2p.tile([128, D], F32, tag=f"p2_{q}", name=f"p2_{q}",
                 padded_shape=[128, 512])
        for q in range(4)
    ]

    for gp in range(G // 2):
        if gp < CGP:
            ws, wt = cws[gp], cwt[gp]
        elif gp in pending:
            ws, wt = pending.pop(gp)
        else:
            ws = wsp.tile([128, 2, IL, JL, 2, R], BF16, tag="ws")
            nc.sync.dma_start(ws[:], wsrc[gp])
            wt = wtp.tile([128, 2, JL, D], BF16, tag="wt")
            nc.scalar.dma_start(wt[:], wtgt[gp])
        for gs in range(2):
            g = 2 * gp + gs
            # einsum1: inter[(il,r), jl, b] = sum_d h[b,4g+il,d] Ws[4g+il,jl,d,r]
            ips = ipsp.tile([128, JL, B], F32, tag="ips")
            for il in range(IL):
                i = IL * g + il
                for jl in range(JL):
                    for dt in range(2):
                        nc.tensor.matmul(
                            ips[32 * il : 32 * (il + 1), jl, :],
                            ws[:, gs, il, jl, dt, :],
                            hT_bf[dt][:, i, :],
                            start=(dt == 0), stop=(dt == 1),
                            tile_position=(0, 32 * il),
                        )
            isb = isbp.tile([128, JL, B], BF16, tag="isb")
            nc.vector.tensor_copy(isb[:], ips[:])

            # einsum2: infl[b,jl,d] += sum_{il,r} inter[(il,r),jl,b] Wt[(il,r),jl,d]
            for jl in range(JL):
                q, m = divmod(jl, 4)
                nc.tensor.matmul(
                    p2[q][32 * m : 32 * m + 8, :],
                    isb[:, jl, :],
                    wt[:, gs, jl, :],
                    start=(g == 0), stop=(g == G - 1),
                    tile_position=(0, 32 * m),
                )

    # prefetch step t+1's first streamed pairs now, in program order BEFORE
    # the AllGather, so the weight stream keeps running through the
    # LN/collective serial chain
    if prefetch_next:
        _prefetch(nc, io, wsp, wtp, pending, 0)

    # relu + residual + LayerNorm on the sparse (slot,batch)-row layout
    # (mean/var reductions fused into the elementwise ops via accum_out)
    hsum = sbp.tile([128, 4, D], F32, tag="hsum")
    msum = sbp.tile([128, 4], F32, tag="msum")
    for q in range(4):
        nc.vector.scalar_tensor_tensor(
            hsum[:, q, :], p2[q][:], 0.0, h_upd[:, q, :],
            op0=mybir.AluOpType.max, op1=mybir.AluOpType.add,
            accum_out=msum[:, q : q + 1],
        )
    mean = sbp.tile([128, 4], F32, tag="mean")
    nc.vector.tensor_scalar_mul(mean[:], msum[:], 1.0 / D)
    cen = sbp.tile([128, 4, D], F32, tag="cen")
    sq = sbp.tile([128, 4, D], F32, tag="sq")
    vsum = sbp.tile([128, 4], F32, tag="vsum")
    nc.vector.tensor_tensor(
        cen[:], hsum[:], mean[:].to_broadcast((128, 4, D)),
        op=mybir.AluOpType.subtract,
    )
    for q in range(4):
        nc.vector.scalar_tensor_tensor(
            sq[:, q, :], cen[:, q, :], 0.0, cen[:, q, :],
            op0=mybir.AluOpType.add, op1=mybir.AluOpType.mult,
            accum_out=vsum[:, q : q + 1],
        )
    std = sbp.tile([128, 4], F32, tag="std")
    nc.scalar.activation(
        std[:], vsum[:], mybir.ActivationFunctionType.Sqrt,
        bias=eps_sb[:], scale=1.0 / D,
    )
    rstd = sbp.tile([128, 4], F32, tag="rstd")
    nc.vector.reciprocal(rstd[:], std[:])
    hnorm = sbp.tile([128, 4, D], F32, tag="hnorm")
    nc.vector.tensor_tensor(
        hnorm[:], cen[:], rstd[:].to_broadcast((128, 4, D)),
        op=mybir.AluOpType.mult,
    )
    g_b = lngb_sb[:, t : t + 1, 0:D].to_broadcast((128, 4, D))
    b_b = lngb_sb[:, t : t + 1, D : 2 * D].to_broadcast((128, 4, D))
    nc.vector.tensor_tensor(hnorm[:], hnorm[:], g_b, op=mybir.AluOpType.mult)
    nc.vector.tensor_tensor(h_upd[:], hnorm[:], b_b, op=mybir.AluOpType.add)

    # transpose updated slots -> bf16 h^T_local; one AllGather per d-half so
    # the first collective overlaps the second half's transposes
    hTl = [
        sbp.tile([128, JL * B], BF16, tag=f"hTl{dt}", name=f"hTl{dt}")
        for dt in range(2)
    ]
    agin = [
        drp.tile([128 * JL * B], BF16, tag=f"agin{dt}", name=f"agin{dt}")
        for dt in range(2)
    ]
    agout = [
        drp.tile([NC, 128 * JL * B], BF16, addr_space="Shared",
                 tag=f"agout{dt}", name=f"agout{dt}")
        for dt in range(2)
    ]
    for dt in range(2):
        for q in range(4):
            p3 = ipsp.tile([128, JL, B], F32, tag="ips")
            p3v = p3[:].rearrange("p jl b -> p (jl b)")
            nc.tensor.transpose(
                p3v, h_upd[:, q, 128 * dt : 128 * (dt + 1)], ident[:]
            )
            nc.vector.tensor_copy(
                hTl[dt][:, 32 * q : 32 * (q + 1)].rearrange(
                    "p (m b) -> p m b", m=4
                ),
                p3v.rearrange("p (m gap b) -> p m gap b", m=4, gap=4)[:, :, 0],
            )
        nc.gpsimd.dma_start(
            agin[dt][:].rearrange("(p f) -> p f", p=128), hTl[dt][:]
        )
        nc.gpsimd.collective_compute(
            "AllGather", mybir.AluOpType.bypass,
            ins=[agin[dt][:].opt()], outs=[agout[dt][:].opt()],
            replica_groups=[list(range(NC))],
        )
        agr = agout[dt][:].rearrange(
            "rk (dp jl b) -> dp rk jl b", dp=128, jl=JL, b=B
        )
        nc.gpsimd.dma_start(
            hT_bf[dt][:].rearrange("dp (rk jl) b -> dp rk jl b", rk=NC), agr
        )


def _phase_c(nc, tc, io, ident, pid, hT_bf, qoT):
    with tc.tile_pool(name="pc_sb", bufs=1) as pc, \
         tc.tile_pool(name="pc_wo", bufs=4) as wop:
        yT_bf = _phase_c_attn(nc, tc, io, ident, pid, hT_bf, qoT, pc)
        _phase_c_vocab(nc, tc, io, yT_bf, wop)


def _phase_c_attn(nc, tc, io, ident, pid, hT_bf, qoT, pc):
    with tc.tile_pool(name="pc_ps", bufs=3, space="PSUM") as cps:

        wkf_sb = [pc.tile([128, D], F32, name=f"wkf{ct}") for ct in range(2)]
        wvf_sb = [pc.tile([128, D], F32, name=f"wvf{ct}") for ct in range(2)]
        for ct in range(2):
            nc.scalar.dma_start(
                wkf_sb[ct][:], io["wkfT"][128 * ct : 128 * (ct + 1), :]
            )
            nc.scalar.dma_start(
                wvf_sb[ct][:], io["wvfT"][128 * ct : 128 * (ct + 1), :]
            )

        # own-batch h^T slice (dynamic b=pid) -> f32 tiles
        pid_v = nc.vector.partition_id()
        hb_bf = [pc.tile([128, S], BF16, name=f"hbb{dt}") for dt in range(2)]
        hb = [pc.tile([128, S], F32, name=f"hb{dt}") for dt in range(2)]
        for dt in range(2):
            nc.vector.tensor_copy(
                hb_bf[dt][:].rearrange("p (s o) -> p s o", o=1),
                hT_bf[dt][:, :, bass.ds(pid_v, 1)],
            )
            nc.vector.tensor_copy(hb[dt][:], hb_bf[dt][:])

        # K_f^T [d'128 x2, s128] ; V_f natural [s, d']
        kfT = [pc.tile([128, S], F32, name=f"kfT{pt}") for pt in range(2)]
        for pt in range(2):
            ps = cps.tile([128, L], F32, tag="c")
            for ct in range(2):
                nc.tensor.matmul(
                    ps[:, 0:S], wkf_sb[ct][:, 128 * pt : 128 * (pt + 1)], hb[ct][:],
                    start=(ct == 0), stop=(ct == 1),
                )
            nc.vector.tensor_copy(kfT[pt][:], ps[:, 0:S])
        vf = pc.tile([S, D], F32)
        psv = cps.tile([128, L], F32, tag="c")
        for ct in range(2):
            nc.tensor.matmul(
                psv[0:S, 0:D], hb[ct][:], wvf_sb[ct][:],
                start=(ct == 0), stop=(ct == 1),
            )
        nc.vector.tensor_copy(vf[:], psv[0:S, 0:D])

        # expand attention -> A2^T [s, t512]
        a2T = pc.tile([S, L], F32)
        for tt in range(4):
            sc = cps.tile([128, L], F32, tag="c")
            for pt in range(2):
                nc.tensor.matmul(
                    sc[:, 0:S], qoT[pt][:, 128 * tt : 128 * (tt + 1)], kfT[pt][:],
                    start=(pt == 0), stop=(pt == 1),
                )
            rowmax = pc.tile([128, 1], F32, tag="rmax2")
            nc.vector.tensor_reduce(
                rowmax[:], sc[:, 0:S], axis=mybir.AxisListType.X,
                op=mybir.AluOpType.max,
            )
            nb = pc.tile([128, 1], F32, tag="nb2")
            nc.vector.tensor_scalar_mul(nb[:], rowmax[:], -SCALE)
            a2 = pc.tile([128, S], F32, tag="a2")
            sumexp = pc.tile([128, 1], F32, tag="sexp2")
            nc.scalar.activation(
                a2[:], sc[:, 0:S], mybir.ActivationFunctionType.Exp,
                bias=nb[:], scale=SCALE, accum_out=sumexp[:],
            )
            rs = pc.tile([128, 1], F32, tag="rs2")
            nc.vector.reciprocal(rs[:], sumexp[:])
            nc.vector.tensor_scalar_mul(a2[:], a2[:], rs[:])
            ptr = cps.tile([128, L], F32, tag="c")
            nc.tensor.transpose(ptr[:, 0:S], a2[:], ident[:])
            nc.vector.tensor_copy(a2T[:, 128 * tt : 128 * (tt + 1)], ptr[:, 0:S])

        # Y^T [d128 x2, t512] -> bf16
        yT_bf = [pc.tile([128, L], BF16, name=f"yTb{dt}") for dt in range(2)]
        for dt in range(2):
            ps = cps.tile([128, L], F32, tag="c")
            nc.tensor.matmul(
                ps[:], vf[:, 128 * dt : 128 * (dt + 1)], a2T[:],
                start=True, stop=True,
            )
            nc.vector.tensor_copy(yT_bf[dt][:], ps[:])
    return yT_bf


def _phase_c_vocab(nc, tc, io, yT_bf, wop):
    # logits tiles (bf16 accumulate in f32 PSUM, bf16 store); deep PSUM
    # pipeline, casts split across Vector and Scalar engines, one batched
    # store per vocab tile so the SP ring isn't trigger-bound

    with tc.tile_pool(name="pc_lg", bufs=6, space="PSUM") as lgps:
        for vp in range(NVT // 2):
            wo_sb = wop.tile([128, 2, 2, VT], BF16, tag="wo")
            nc.scalar.dma_start(
                wo_sb[:],
                io["woutT"][2 * vp : 2 * vp + 2].rearrange(
                    "v2 dt dp v -> dp v2 dt v"
                ),
            )
            for v2 in range(2):
                vt = 2 * vp + v2
                lg_sb = wop.tile([128, 4, VT], BF16, tag="lg_sb", name="lg_sb")
                for tt in range(4):
                    lg = lgps.tile([128, VT], F32, tag="lg")
                    for dt in range(2):
                        nc.tensor.matmul(
                            lg[:],
                            yT_bf[dt][:, 128 * tt : 128 * (tt + 1)],
                            wo_sb[:, v2, dt, :],
                            start=(dt == 0), stop=(dt == 1),
                        )
                    if tt % 2 == 0:
                        nc.vector.tensor_copy(lg_sb[:, tt, :], lg[:])
                    else:
                        nc.scalar.copy(lg_sb[:, tt, :], lg[:])
                nc.sync.dma_start(io["lg_out"][vt], lg_sb[:])


# ---------------------------------------------------------------------------
# Host side
# ---------------------------------------------------------------------------

_NC_CACHE = {}


def _get_nc():
    key = (N_STEPS, WBUFS)
    if key not in _NC_CACHE:
        _NC_CACHE[key] = build()
    return _NC_CACHE[key]


def _prep_in_maps(inputs):
    f32 = lambda a: np.ascontiguousarray(np.asarray(a), dtype=np.float32)
    input_ids = np.asarray(inputs["input_ids"])
    attention_mask = np.asarray(inputs["attention_mask"])
    H = f32(inputs["H"])
    W_source = f32(inputs["W_source"])
    W_target = f32(inputs["W_target"])

    lngb = np.zeros((128, STEPS, 2 * D), dtype=np.float32)
    lngb[:, :, 0:D] = np.asarray(inputs["ln_scale"])[None]
    lngb[:, :, D:] = np.asarray(inputs["ln_bias"])[None]

    rep = {
        "wqT": f32(np.asarray(inputs["Wq_in"]).T),
        "wkslT": f32(np.asarray(inputs["Wk_slots"]).T),
        "wvT": f32(np.asarray(inputs["Wv_in"]).T),
        "wqoT": f32(np.asarray(inputs["Wq_out"]).T),
        "wkfT": f32(np.asarray(inputs["Wk_fin"]).T),
        "wvfT": f32(np.asarray(inputs["Wv_fin"]).T),
        "hT_in": f32(H.T),
        "hn_in": H,
        "lngb": lngb,
        # woutT[vt, dtile, dp, vl] = Wout[500vt+vl, 128dt+dp]
        "woutT": np.ascontiguousarray(
            f32(inputs["W_out_proj"]).reshape(NVT, VT, 2, 128).transpose(0, 2, 3, 1)
        ).astype(BF16_NP),
    }

    tok = np.asarray(inputs["token_emb"], dtype=np.float32)
    pos = np.asarray(inputs["pos_emb"], dtype=np.float32)

    in_maps = []
    for c in range(NC):
        m = dict(rep)
        X = tok[input_ids[c]] + pos
        m["xT_in"] = np.ascontiguousarray(X.T)
        m["maskw"] = np.ascontiguousarray(
            attention_mask[c].astype(np.float32).reshape(4, 128).T
        )
        # wsrc[gp, dp, gs, il, jl, dt, r] = W_source[8gp+4gs+il, 16c+jl, 128dt+dp, r]
        ws = W_source[:, JL * c : JL * (c + 1)]          # [S, 16, D, R]
        ws = ws.reshape(G // 2, 2, IL, JL, 2, 128, R).transpose(0, 5, 1, 2, 3, 4, 6)
        m["wsrc"] = np.ascontiguousarray(ws).astype(BF16_NP)
        # wtgt[gp, (il r), gs, jl, d] = W_target[8gp+4gs+il, 16c+jl, r, d]
        # The reference masks out the i == j (diagonal) pair; zeroing
        # W_target[j, j] is exactly equivalent since the term is linear in it.
        wt = W_target[:, JL * c : JL * (c + 1)].copy()   # [S, 16, R, D]
        for jl in range(JL):
            wt[JL * c + jl, jl] = 0.0
        wt = wt.reshape(G // 2, 2, IL, JL, R, D).transpose(0, 2, 4, 1, 3, 5)
        m["wtgt"] = np.ascontiguousarray(
            wt.reshape(G // 2, 128, 2, JL, D)
        ).astype(BF16_NP)
        in_maps.append(m)
    return in_maps


def run(inputs, trace=False):
    nc = _get_nc()
    in_maps = _prep_in_maps(inputs)
    res = bass_utils.run_bass_kernel_spmd(
        nc, in_maps, core_ids=list(range(NC)), trace=trace
    )
    out = np.stack(
        [
            np.asarray(res.results[c]["lg_out"], dtype=np.float32)
            .transpose(2, 1, 0, 3)
            .reshape(L, V)
            for c in range(NC)
        ],
        axis=0,
    )
    return out, res


def kernel(**inputs):
    out, _ = run(inputs, trace=False)
    return out


# revision 39
# speedup vs baseline: 1.1201x; 1.1201x over previous
"""Trainium2 Bass kernel for nn_ConnectionTransformer (8 NeuronCores, SPMD).

Strategy (v2 — bf16 weight streaming)
-------------------------------------
- Phase A (embed + compress attention): batch-parallel, core c handles batch c.
- Phase B (6 bilinear message-passing steps): target-slot sharding — core c owns
  16 target slots j in [16c, 16c+16). The per-pair weights W_source/W_target are
  converted to bf16 on the host (1 GB -> 512 MB total, j-sharded 8 ways) and
  streamed from HBM once per step per core — the memory roofline.
  Source slots are processed in groups g of IL=4 so the second einsum contracts
  over a full 128-deep (il, r) partition axis in a single matmul per target
  slot.  State h is kept both as bf16 h^T (matmul operand) and as an f32
  "sparse" (slot,batch)-row layout for relu/residual/LayerNorm.  A bf16
  AllGather rebuilds the replicated h^T each step.
- Phase C (expand attention + vocab projection): batch-parallel; W_out and the
  logits are bf16 (host casts logits back to f32).
"""
import os
import sys

sys.path.insert(0, "/opt/trn_rl_repo")

import numpy as np
from concourse import bass, bacc, tile, bass_utils, mybir
from concourse import masks

B, L, D, S, R, STEPS, V = 8, 512, 256, 128, 32, 6, 32000
NC = 8
JL = S // NC          # 16 local target slots per core
IL = 4                # source slots per group
G = S // IL           # 32 source groups
VT = 500              # vocab tile width
NVT = V // VT         # 64 vocab tiles
SCALE = 1.0 / np.sqrt(D)
LN_EPS = 1e-5

F32 = mybir.dt.float32
BF16 = mybir.dt.bfloat16
BF16_NP = mybir.dt.np(mybir.dt.bfloat16)

N_STEPS = int(os.environ.get("N_STEPS", str(STEPS)))
WBUFS = int(os.environ.get("WBUFS", "3"))
CGP = int(os.environ.get("CGP", "2"))


# ---------------------------------------------------------------------------
# Device program
# ---------------------------------------------------------------------------

def build():
    nc = bacc.Bacc("TRN2", target_bir_lowering=False, debug=False, num_devices=NC)

    io = {}

    def inp(name, shape, dtype=F32):
        io[name] = nc.dram_tensor(name, shape, dtype, kind="ExternalInput").ap()

    inp("xT_in", [D, L])
    inp("maskw", [128, 4])
    for w in ("wqT", "wkslT", "wvT", "wqoT", "wkfT", "wvfT"):
        inp(w, [D, D])
    inp("hT_in", [D, S])
    inp("hn_in", [S, D])
    inp("lngb", [128, STEPS, 2 * D])
    inp("wsrc", [G // 2, 128, 2, IL, JL, 2, R], BF16)
    inp("wtgt", [G // 2, 128, 2, JL, D], BF16)
    inp("woutT", [NVT, 2, 128, VT], BF16)
    io["lg_out"] = nc.dram_tensor(
        "lg_out", [NVT, 128, 4, VT], BF16, kind="ExternalOutput"
    ).ap()

    with tile.TileContext(nc) as tc:
        _body(nc, tc, io)
    nc.compile()
    return nc


def _body(nc, tc, io):
    with tc.tile_pool(name="const", bufs=1) as const, \
         tc.tile_pool(name="state", bufs=1) as state, \
         tc.tile_pool(name="isbp", bufs=3) as isbp, \
         tc.tile_pool(name="sbp", bufs=1) as sbp, \
         tc.tile_pool(name="drp", bufs=2, space="DRAM") as drp:

        ident = const.tile([128, 128], F32)
        masks.make_identity(nc, ident[:])
        ones = const.tile([128, 1], F32)
        nc.vector.memset(ones[:], 1.0)
        eps_sb = const.tile([128, 1], F32)
        nc.vector.memset(eps_sb[:], LN_EPS)

        pid = nc.sync.partition_id()
        gpid = nc.gpsimd.partition_id()

        # tiny dummy AllGather to absorb the first-collective bringup latency
        # in parallel with phase A
        wup_in = drp.tile([32], F32, tag="wup_in")
        wup_out = drp.tile([NC, 32], F32, addr_space="Shared", tag="wup_out")
        nc.gpsimd.collective_compute(
            "AllGather", mybir.AluOpType.bypass,
            ins=[wup_in[:].opt()], outs=[wup_out[:].opt()],
            replica_groups=[list(range(NC))],
        )

        # persistent state
        # h^T for matmuls: [dp, slot, batch] bf16, one tile per 128-d half
        hT_bf = [state.tile([128, S, B], BF16, name=f"hTbf{dt}") for dt in range(2)]
        # own slots, natural layout, f32, "sparse" rows: partition 32m+b holds
        # slot jl=4q+m of batch b at free index q  (rows 8..31 of each 32-block
        # are unused garbage)
        h_upd = state.tile([128, 4, D], F32)
        qoT = [state.tile([128, L], F32, name=f"qoT{pt}") for pt in range(2)]
        lngb_sb = state.tile([128, STEPS, 2 * D], F32)
        nc.gpsimd.dma_start(lngb_sb[:], io["lngb"])

        # weight streaming pools live through phase A (prefetch) + the steps,
        # released before phase C so its pools get the SBUF space back
        with tc.tile_pool(name="wsp", bufs=WBUFS) as wsp, \
             tc.tile_pool(name="wtp", bufs=WBUFS) as wtp:
            _phase_a(nc, tc, io, ident, ones, pid, hT_bf, h_upd, qoT)
            # persistent cache for the first CGP g-pairs: loaded once (after
            # phase A frees its SBUF), reused by all steps.  Cached pairs are
            # the step head, so compute restarts right after each AllGather
            # without waiting on weight DMA.
            with tc.tile_pool(name="wcache", bufs=1) as wc, \
                 tc.tile_pool(name="ipsp", bufs=2, space="PSUM") as ipsp, \
                 tc.tile_pool(name="p2p", bufs=1, space="PSUM") as p2p:
                cws, cwt = [], []
                for k in range(CGP):
                    c_ws = wc.tile([128, 2, IL, JL, 2, R], BF16, name=f"cws{k}")
                    nc.sync.dma_start(c_ws[:], io["wsrc"][k])
                    cws.append(c_ws)
                    c_wt = wc.tile([128, 2, JL, D], BF16, name=f"cwt{k}")
                    nc.scalar.dma_start(c_wt[:], io["wtgt"][k])
                    cwt.append(c_wt)
                # streamed-weight prefetch handoff: tiles DMA'd before step
                # t's AllGather, consumed at the head of step t+1
                pending = {}
                _prefetch(nc, io, wsp, wtp, pending, 0)
                for t in range(N_STEPS):
                    _step(nc, tc, t, io, hT_bf, h_upd, lngb_sb, ident, eps_sb,
                          wsp, wtp, isbp, sbp, ipsp, p2p, drp, cws, cwt,
                          pending, (t + 1 < N_STEPS), gpid)
        _phase_c(nc, tc, io, ident, pid, hT_bf, qoT)


def _phase_a(nc, tc, io, ident, ones, pid, hT_bf, h_upd, qoT):
    with tc.tile_pool(name="pa_sb", bufs=1) as pa, \
         tc.tile_pool(name="pa_ps", bufs=3, space="PSUM") as pps, \
         tc.tile_pool(name="pa_tp", bufs=2, space="PSUM") as tps, \
         tc.tile_pool(name="pa_acc", bufs=1, space="PSUM") as aps, \
         tc.tile_pool(name="dram_a", bufs=1, space="DRAM") as dra:

        mask_sb = pa.tile([128, 4], F32)
        nc.gpsimd.dma_start(mask_sb[:], io["maskw"])

        # X^T tiles [d128, t512] (host-gathered embeddings, transposed)
        xT = [pa.tile([128, L], F32, name=f"xT{ct}") for ct in range(2)]
        for ct in range(2):
            nc.gpsimd.dma_start(xT[ct][:], io["xT_in"][128 * ct : 128 * (ct + 1), :])

        # weight tiles [d128, 256] (contraction on partitions)
        def load_w(name):
            ts = [pa.tile([128, D], F32, name=f"{name}_{ct}") for ct in range(2)]
            for ct in range(2):
                nc.gpsimd.dma_start(ts[ct][:], io[name][128 * ct : 128 * (ct + 1), :])
            return ts

        wq_sb = load_w("wqT")
        wv_sb = load_w("wvT")
        wksl_sb = load_w("wkslT")
        wqo_sb = load_w("wqoT")
        hTt = [pa.tile([128, S], F32, name=f"hTt{ct}") for ct in range(2)]
        for ct in range(2):
            nc.gpsimd.dma_start(hTt[ct][:], io["hT_in"][128 * ct : 128 * (ct + 1), :])
        hn_sb = pa.tile([S, D], F32)
        nc.gpsimd.dma_start(hn_sb[:], io["hn_in"])

        # Q_in^T and Q_out^T : [d'128 x 2, t512]
        qT = [pa.tile([128, L], F32, name=f"qT{pt}") for pt in range(2)]
        for pt in range(2):
            for dst, wsb in ((qT, wq_sb), (qoT, wqo_sb)):
                ps = pps.tile([128, L], F32, tag="ps")
                for ct in range(2):
                    nc.tensor.matmul(
                        ps[:], wsb[ct][:, 128 * pt : 128 * (pt + 1)], xT[ct][:],
                        start=(ct == 0), stop=(ct == 1),
                    )
                nc.vector.tensor_copy(dst[pt][:], ps[:])

        # V_in natural [t128 x 4, d256]
        vn = pa.tile([128, 4, D], F32)
        for tt in range(4):
            ps = pps.tile([128, L], F32, tag="ps")
            for ct in range(2):
                nc.tensor.matmul(
                    ps[:, 0:D], xT[ct][:, 128 * tt : 128 * (tt + 1)], wv_sb[ct][:],
                    start=(ct == 0), stop=(ct == 1),
                )
            nc.vector.tensor_copy(vn[:, tt, :], ps[:, 0:D])

        # K_slots^T [d'128 x 2, s128]
        kslT = [pa.tile([128, S], F32, name=f"kslT{pt}") for pt in range(2)]
        for pt in range(2):
            ps = pps.tile([128, L], F32, tag="ps")
            for ct in range(2):
                nc.tensor.matmul(
                    ps[:, 0:S], wksl_sb[ct][:, 128 * pt : 128 * (pt + 1)], hTt[ct][:],
                    start=(ct == 0), stop=(ct == 1),
                )
            nc.vector.tensor_copy(kslT[pt][:], ps[:, 0:S])

        # attention scores + masked softmax
        a_sb = pa.tile([128, 4, S], F32)
        for tt in range(4):
            sc = pps.tile([128, L], F32, tag="ps")
            for pt in range(2):
                nc.tensor.matmul(
                    sc[:, 0:S], qT[pt][:, 128 * tt : 128 * (tt + 1)], kslT[pt][:],
                    start=(pt == 0), stop=(pt == 1),
                )
            rowmax = pa.tile([128, 1], F32, tag="rmax")
            nc.vector.tensor_reduce(
                rowmax[:], sc[:, 0:S], axis=mybir.AxisListType.X,
                op=mybir.AluOpType.max,
            )
            nb = pa.tile([128, 1], F32, tag="nb")
            nc.vector.tensor_scalar_mul(nb[:], rowmax[:], -SCALE)
            sumexp = pa.tile([128, 1], F32, tag="sexp")
            nc.scalar.activation(
                a_sb[:, tt, :], sc[:, 0:S], mybir.ActivationFunctionType.Exp,
                bias=nb[:], scale=SCALE, accum_out=sumexp[:],
            )
            rs = pa.tile([128, 1], F32, tag="rs")
            nc.vector.reciprocal(rs[:], sumexp[:])
            rm = pa.tile([128, 1], F32, tag="rmk")
            nc.vector.tensor_tensor(
                rm[:], rs[:], mask_sb[:, tt : tt + 1], op=mybir.AluOpType.mult
            )
            nc.vector.tensor_scalar_mul(a_sb[:, tt, :], a_sb[:, tt, :], rm[:])

        # column sums and IR = A^T @ V
        cs = aps.tile([128, 1], F32, tag="cs")
        for tt in range(4):
            nc.tensor.matmul(
                cs[:], a_sb[:, tt, :], ones[:, 0:1], start=(tt == 0), stop=(tt == 3)
            )
        ir = aps.tile([128, D], F32, tag="ir")
        for tt in range(4):
            nc.tensor.matmul(
                ir[:], a_sb[:, tt, :], vn[:, tt, :], start=(tt == 0), stop=(tt == 3)
            )
        cssb = pa.tile([128, 1], F32)
        nc.vector.tensor_scalar_add(cssb[:], cs[:], 1e-8)
        rcs = pa.tile([128, 1], F32)
        nc.vector.reciprocal(rcs[:], cssb[:])
        h0 = pa.tile([S, D], F32)
        nc.vector.scalar_tensor_tensor(
            h0[:], ir[:], rcs[:], hn_sb[:],
            op0=mybir.AluOpType.mult, op1=mybir.AluOpType.add,
        )

        # h0 natural bounce -> AllGather over batches (f32, 128 KB per core)
        agin0 = dra.tile([S * D], F32)
        nc.gpsimd.dma_start(
            agin0[:].rearrange("(p f) -> p f", p=128), h0[:]
        )
        agout0 = dra.tile([NC, S * D], F32, addr_space="Shared")
        nc.gpsimd.collective_compute(
            "AllGather", mybir.AluOpType.bypass,
            ins=[agin0[:].opt()], outs=[agout0[:].opt()],
            replica_groups=[list(range(NC))],
        )
        # readback per batch (contiguous), transpose on the PE, cast into the
        # bf16 h^T state.  hT_bf position p holds global slot (16c + p) mod S
        # (core-local rotation; matches the host-rotated wsrc/wtgt order), so
        # each 16-slot block is copied from a pid-dependent source offset.
        pid_v = nc.vector.partition_id()
        for b in range(B):
            hnat = pa.tile([S, D], F32, tag="hnat", name="hnat", bufs=2)
            nc.gpsimd.dma_start(
                hnat[:], agout0[b].rearrange("(s d) -> s d", d=D)
            )
            for dt in range(2):
                p3 = tps.tile([128, 128], F32, tag="tp")
                nc.tensor.transpose(
                    p3[:], hnat[:, 128 * dt : 128 * (dt + 1)], ident[:]
                )
                for k in range(NC):
                    nc.vector.tensor_copy(
                        hT_bf[dt][:, JL * k : JL * (k + 1), b : b + 1],
                        p3[:, bass.ds(JL * ((pid_v + k) % NC), JL)],
                    )
        # own 16 slots (all batches) into the sparse f32 residual layout:
        # h_upd[32m+b, q, d] = h0_b[slot 16c+4q+m, d]
        nat = agout0[:][:, bass.ds(pid * (JL * D), JL * D)]
        natr = nat.rearrange("b (q m d) -> m b q d", q=4, m=4, d=D)
        for m in range(4):
            nc.sync.dma_start(h_upd[32 * m : 32 * m + B, :, :], natr[m])


def _prefetch(nc, io, wsp, wtp, pending, _unused):
    """DMA the first WBUFS streamed g-pairs of the next step into fresh pool
    buffers; the next _step call picks them up from `pending`."""
    for gp in range(CGP, min(CGP + WBUFS, G // 2)):
        ws = wsp.tile([128, 2, IL, JL, 2, R], BF16, tag="ws", name="ws_pf")
        nc.sync.dma_start(ws[:], io["wsrc"][gp])
        wt = wtp.tile([128, 2, JL, D], BF16, tag="wt", name="wt_pf")
        nc.scalar.dma_start(wt[:], io["wtgt"][gp])
        pending[gp] = (ws, wt)


def _step(nc, tc, t, io, hT_bf, h_upd, lngb_sb, ident, eps_sb,
          wsp, wtp, isbp, sbp, ipsp, p2p, drp, cws, cwt, pending,
          prefetch_next, gpid):
    """One message-passing step."""
    wsrc, wtgt = io["wsrc"], io["wtgt"]
    # influence accumulators: partition 32m+b of tile q holds slot jl=4q+m.
    # One full (padded) PSUM bank per q so each (q, m) accumulation group owns
    # its whole 2KB zero region.
    p2 = [
        p2p.tile([128, D], F32, tag=f"p2_{q}", name=f"p2_{q}",
                 padded_shape=[128, 512])
        for q in range(4)
    ]

    for gp in range(G // 2):
        if gp < CGP:
            ws, wt = cws[gp], cwt[gp]
        elif gp in pending:
            ws, wt = pending.pop(gp)
        else:
            ws = wsp.tile([128, 2, IL, JL, 2, R], BF16, tag="ws")
            nc.sync.dma_start(ws[:], wsrc[gp])
            wt = wtp.tile([128, 2, JL, D], BF16, tag="wt")
            nc.scalar.dma_start(wt[:], wtgt[gp])
        for gs in range(2):
            g = 2 * gp + gs
            # einsum1: inter[(il,r), jl, b] = sum_d h[b,4g+il,d] Ws[4g+il,jl,d,r]
            ips = ipsp.tile([128, JL, B], F32, tag="ips")
            for il in range(IL):
                i = IL * g + il
                for jl in range(JL):
                    for dt in range(2):
                        nc.tensor.matmul(
                            ips[32 * il : 32 * (il + 1), jl, :],
                            ws[:, gs, il, jl, dt, :],
                            hT_bf[dt][:, i, :],
                            start=(dt == 0), stop=(dt == 1),
                            tile_position=(0, 32 * il),
                        )
            isb = isbp.tile([128, JL, B], BF16, tag="isb")
            nc.vector.tensor_copy(isb[:], ips[:])

            # einsum2: infl[b,jl,d] += sum_{il,r} inter[(il,r),jl,b] Wt[(il,r),jl,d]
            for jl in range(JL):
                q, m = divmod(jl, 4)
                nc.tensor.matmul(
                    p2[q][32 * m : 32 * m + 8, :],
                    isb[:, jl, :],
                    wt[:, gs, jl, :],
                    start=(g == 0), stop=(g == G - 1),
                    tile_position=(0, 32 * m),
                )

    # prefetch step t+1's first streamed pairs now, in program order BEFORE
    # the AllGather, so the weight stream keeps running through the
    # LN/collective serial chain
    if prefetch_next:
        _prefetch(nc, io, wsp, wtp, pending, 0)

    # relu + residual + LayerNorm on the sparse (slot,batch)-row layout
    # (mean/var reductions fused into the elementwise ops via accum_out)
    hsum = sbp.tile([128, 4, D], F32, tag="hsum")
    msum = sbp.tile([128, 4], F32, tag="msum")
    for q in range(4):
        nc.vector.scalar_tensor_tensor(
            hsum[:, q, :], p2[q][:], 0.0, h_upd[:, q, :],
            op0=mybir.AluOpType.max, op1=mybir.AluOpType.add,
            accum_out=msum[:, q : q + 1],
        )
    mean = sbp.tile([128, 4], F32, tag="mean")
    nc.vector.tensor_scalar_mul(mean[:], msum[:], 1.0 / D)
    cen = sbp.tile([128, 4, D], F32, tag="cen")
    sq = sbp.tile([128, 4, D], F32, tag="sq")
    vsum = sbp.tile([128, 4], F32, tag="vsum")
    nc.vector.tensor_tensor(
        cen[:], hsum[:], mean[:].to_broadcast((128, 4, D)),
        op=mybir.AluOpType.subtract,
    )
    for q in range(4):
        nc.vector.scalar_tensor_tensor(
            sq[:, q, :], cen[:, q, :], 0.0, cen[:, q, :],
            op0=mybir.AluOpType.add, op1=mybir.AluOpType.mult,
            accum_out=vsum[:, q : q + 1],
        )
    std = sbp.tile([128, 4], F32, tag="std")
    nc.scalar.activation(
        std[:], vsum[:], mybir.ActivationFunctionType.Sqrt,
        bias=eps_sb[:], scale=1.0 / D,
    )
    rstd = sbp.tile([128, 4], F32, tag="rstd")
    nc.vector.reciprocal(rstd[:], std[:])
    hnorm = sbp.tile([128, 4, D], F32, tag="hnorm")
    nc.vector.tensor_tensor(
        hnorm[:], cen[:], rstd[:].to_broadcast((128, 4, D)),
        op=mybir.AluOpType.mult,
    )
    g_b = lngb_sb[:, t : t + 1, 0:D].to_broadcast((128, 4, D))
    b_b = lngb_sb[:, t : t + 1, D : 2 * D].to_broadcast((128, 4, D))
    nc.vector.tensor_tensor(hnorm[:], hnorm[:], g_b, op=mybir.AluOpType.mult)
    nc.vector.tensor_tensor(h_upd[:], hnorm[:], b_b, op=mybir.AluOpType.add)

    # transpose updated slots -> bf16 h^T_local; one AllGather per d-half so
    # the first collective overlaps the second half's transposes
    hTl = [
        sbp.tile([128, JL * B], BF16, tag=f"hTl{dt}", name=f"hTl{dt}")
        for dt in range(2)
    ]
    agin = [
        drp.tile([128 * JL * B], BF16, tag=f"agin{dt}", name=f"agin{dt}")
        for dt in range(2)
    ]
    agout = [
        drp.tile([NC, 128 * JL * B], BF16, addr_space="Shared",
                 tag=f"agout{dt}", name=f"agout{dt}")
        for dt in range(2)
    ]
    for dt in range(2):
        for q in range(4):
            p3 = ipsp.tile([128, JL, B], F32, tag="ips")
            p3v = p3[:].rearrange("p jl b -> p (jl b)")
            nc.tensor.transpose(
                p3v, h_upd[:, q, 128 * dt : 128 * (dt + 1)], ident[:]
            )
            nc.vector.tensor_copy(
                hTl[dt][:, 32 * q : 32 * (q + 1)].rearrange(
                    "p (m b) -> p m b", m=4
                ),
                p3v.rearrange("p (m gap b) -> p m gap b", m=4, gap=4)[:, :, 0],
            )
        # own slots live at positions 0..JL: write them locally so the cached
        # head pairs (own-slot weights) can start before the AllGather lands
        nc.vector.tensor_copy(
            hT_bf[dt][:, 0:JL, :],
            hTl[dt][:].rearrange("p (jl b) -> p jl b", jl=JL),
        )
        nc.gpsimd.dma_start(
            agin[dt][:].rearrange("(p f) -> p f", p=128), hTl[dt][:]
        )
        nc.gpsimd.collective_compute(
            "AllGather", mybir.AluOpType.bypass,
            ins=[agin[dt][:].opt()], outs=[agout[dt][:].opt()],
            replica_groups=[list(range(NC))],
        )
        # rotated readback: position block k holds core (c+k) mod 8's slots;
        # static destination ranges keep the cached-head reads dependency-free
        agof = agout[dt][:].rearrange("r f -> (r f)")
        for k in range(1, NC):
            src_k = agof[
                bass.ds(JL * B * 128 * ((gpid + k) % NC), JL * B * 128)
            ].rearrange("(dp f) -> dp f", dp=128)
            nc.gpsimd.dma_start(
                hT_bf[dt][:, JL * k : JL * (k + 1), :].rearrange(
                    "dp jl b -> dp (jl b)"
                ),
                src_k,
            )


def _phase_c(nc, tc, io, ident, pid, hT_bf, qoT):
    with tc.tile_pool(name="pc_sb", bufs=1) as pc, \
         tc.tile_pool(name="pc_wo", bufs=6) as wop:
        yT_bf = _phase_c_attn(nc, tc, io, ident, pid, hT_bf, qoT, pc)
        _phase_c_vocab(nc, tc, io, yT_bf, wop)


def _phase_c_attn(nc, tc, io, ident, pid, hT_bf, qoT, pc):
    with tc.tile_pool(name="pc_ps", bufs=3, space="PSUM") as cps:

        wkf_sb = [pc.tile([128, D], F32, name=f"wkf{ct}") for ct in range(2)]
        wvf_sb = [pc.tile([128, D], F32, name=f"wvf{ct}") for ct in range(2)]
        for ct in range(2):
            nc.scalar.dma_start(
                wkf_sb[ct][:], io["wkfT"][128 * ct : 128 * (ct + 1), :]
            )
            nc.scalar.dma_start(
                wvf_sb[ct][:], io["wvfT"][128 * ct : 128 * (ct + 1), :]
            )

        # own-batch h^T slice (dynamic b=pid) -> f32 tiles
        pid_v = nc.vector.partition_id()
        hb_bf = [pc.tile([128, S], BF16, name=f"hbb{dt}") for dt in range(2)]
        hb = [pc.tile([128, S], F32, name=f"hb{dt}") for dt in range(2)]
        for dt in range(2):
            nc.vector.tensor_copy(
                hb_bf[dt][:].rearrange("p (s o) -> p s o", o=1),
                hT_bf[dt][:, :, bass.ds(pid_v, 1)],
            )
            nc.vector.tensor_copy(hb[dt][:], hb_bf[dt][:])

        # K_f^T [d'128 x2, s128] ; V_f natural [s, d']
        kfT = [pc.tile([128, S], F32, name=f"kfT{pt}") for pt in range(2)]
        for pt in range(2):
            ps = cps.tile([128, L], F32, tag="c")
            for ct in range(2):
                nc.tensor.matmul(
                    ps[:, 0:S], wkf_sb[ct][:, 128 * pt : 128 * (pt + 1)], hb[ct][:],
                    start=(ct == 0), stop=(ct == 1),
                )
            nc.vector.tensor_copy(kfT[pt][:], ps[:, 0:S])
        vf = pc.tile([S, D], F32)
        psv = cps.tile([128, L], F32, tag="c")
        for ct in range(2):
            nc.tensor.matmul(
                psv[0:S, 0:D], hb[ct][:], wvf_sb[ct][:],
                start=(ct == 0), stop=(ct == 1),
            )
        nc.vector.tensor_copy(vf[:], psv[0:S, 0:D])

        # expand attention -> A2^T [s, t512]
        a2T = pc.tile([S, L], F32)
        for tt in range(4):
            sc = cps.tile([128, L], F32, tag="c")
            for pt in range(2):
                nc.tensor.matmul(
                    sc[:, 0:S], qoT[pt][:, 128 * tt : 128 * (tt + 1)], kfT[pt][:],
                    start=(pt == 0), stop=(pt == 1),
                )
            rowmax = pc.tile([128, 1], F32, tag="rmax2")
            nc.vector.tensor_reduce(
                rowmax[:], sc[:, 0:S], axis=mybir.AxisListType.X,
                op=mybir.AluOpType.max,
            )
            nb = pc.tile([128, 1], F32, tag="nb2")
            nc.vector.tensor_scalar_mul(nb[:], rowmax[:], -SCALE)
            a2 = pc.tile([128, S], F32, tag="a2")
            sumexp = pc.tile([128, 1], F32, tag="sexp2")
            nc.scalar.activation(
                a2[:], sc[:, 0:S], mybir.ActivationFunctionType.Exp,
                bias=nb[:], scale=SCALE, accum_out=sumexp[:],
            )
            rs = pc.tile([128, 1], F32, tag="rs2")
            nc.vector.reciprocal(rs[:], sumexp[:])
            nc.vector.tensor_scalar_mul(a2[:], a2[:], rs[:])
            ptr = cps.tile([128, L], F32, tag="c")
            nc.tensor.transpose(ptr[:, 0:S], a2[:], ident[:])
            nc.vector.tensor_copy(a2T[:, 128 * tt : 128 * (tt + 1)], ptr[:, 0:S])

        # Y^T [d128 x2, t512] -> bf16
        yT_bf = [pc.tile([128, L], BF16, name=f"yTb{dt}") for dt in range(2)]
        for dt in range(2):
            ps = cps.tile([128, L], F32, tag="c")
            nc.tensor.matmul(
                ps[:], vf[:, 128 * dt : 128 * (dt + 1)], a2T[:],
                start=True, stop=True,
            )
            nc.vector.tensor_copy(yT_bf[dt][:], ps[:])
    return yT_bf


def _phase_c_vocab(nc, tc, io, yT_bf, wop):
    # logits tiles (bf16 accumulate in f32 PSUM, bf16 store); deep PSUM
    # pipeline, casts split across Vector and Scalar engines, one batched
    # store per vocab tile so the SP ring isn't trigger-bound

    with tc.tile_pool(name="pc_lg", bufs=6, space="PSUM") as lgps:
        for vp in range(NVT // 2):
            wo_sb = wop.tile([128, 2, 2, VT], BF16, tag="wo")
            nc.scalar.dma_start(
                wo_sb[:],
                io["woutT"][2 * vp : 2 * vp + 2].rearrange(
                    "v2 dt dp v -> dp v2 dt v"
                ),
            )
            for v2 in range(2):
                vt = 2 * vp + v2
                lg_sb = wop.tile([128, 4, VT], BF16, tag="lg_sb", name="lg_sb")
                for tt in range(4):
                    lg = lgps.tile([128, VT], F32, tag="lg")
                    for dt in range(2):
                        nc.tensor.matmul(
                            lg[:],
                            yT_bf[dt][:, 128 * tt : 128 * (tt + 1)],
                            wo_sb[:, v2, dt, :],
                            start=(dt == 0), stop=(dt == 1),
                        )
                    if tt % 2 == 0:
                        nc.vector.tensor_copy(lg_sb[:, tt, :], lg[:])
                    else:
                        nc.scalar.copy(lg_sb[:, tt, :], lg[:])
                nc.sync.dma_start(io["lg_out"][vt], lg_sb[:])


# ---------------------------------------------------------------------------
# Host side
# ---------------------------------------------------------------------------

_NC_CACHE = {}


def _get_nc():
    key = (N_STEPS, WBUFS)
    if key not in _NC_CACHE:
        _NC_CACHE[key] = build()
    return _NC_CACHE[key]


def _prep_in_maps(inputs):
    f32 = lambda a: np.ascontiguousarray(np.asarray(a), dtype=np.float32)
    input_ids = np.asarray(inputs["input_ids"])
    attention_mask = np.asarray(inputs["attention_mask"])
    H = f32(inputs["H"])
    W_source = f32(inputs["W_source"])
    W_target = f32(inputs["W_target"])

    lngb = np.zeros((128, STEPS, 2 * D), dtype=np.float32)
    lngb[:, :, 0:D] = np.asarray(inputs["ln_scale"])[None]
    lngb[:, :, D:] = np.asarray(inputs["ln_bias"])[None]

    rep = {
        "wqT": f32(np.asarray(inputs["Wq_in"]).T),
        "wkslT": f32(np.asarray(inputs["Wk_slots"]).T),
        "wvT": f32(np.asarray(inputs["Wv_in"]).T),
        "wqoT": f32(np.asarray(inputs["Wq_out"]).T),
        "wkfT": f32(np.asarray(inputs["Wk_fin"]).T),
        "wvfT": f32(np.asarray(inputs["Wv_fin"]).T),
        "hT_in": f32(H.T),
        "hn_in": H,
        "lngb": lngb,
        # woutT[vt, dtile, dp, vl] = Wout[500vt+vl, 128dt+dp]
        "woutT": np.ascontiguousarray(
            f32(inputs["W_out_proj"]).reshape(NVT, VT, 2, 128).transpose(0, 2, 3, 1)
        ).astype(BF16_NP),
    }

    tok = np.asarray(inputs["token_emb"], dtype=np.float32)
    pos = np.asarray(inputs["pos_emb"], dtype=np.float32)

    in_maps = []
    for c in range(NC):
        m = dict(rep)
        X = tok[input_ids[c]] + pos
        m["xT_in"] = np.ascontiguousarray(X.T)
        m["maskw"] = np.ascontiguousarray(
            attention_mask[c].astype(np.float32).reshape(4, 128).T
        )
        # Source slots are processed in core-local rotated order:
        # logical i maps to global slot (16c + i) mod 128, so pairs 0-1
        # (the SBUF-cached step head) are the core's own slots and need no
        # AllGather data.
        # wsrc[gp, dp, gs, il, jl, dt, r] = W_source[rot(8gp+4gs+il), 16c+jl, ...]
        ws = W_source[:, JL * c : JL * (c + 1)]          # [S, 16, D, R]
        ws = np.roll(ws, -JL * c, axis=0)
        ws = ws.reshape(G // 2, 2, IL, JL, 2, 128, R).transpose(0, 5, 1, 2, 3, 4, 6)
        m["wsrc"] = np.ascontiguousarray(ws).astype(BF16_NP)
        # wtgt[gp, (il r), gs, jl, d] = W_target[8gp+4gs+il, 16c+jl, r, d]
        # The reference masks out the i == j (diagonal) pair; zeroing
        # W_target[j, j] is exactly equivalent since the term is linear in it.
        wt = W_target[:, JL * c : JL * (c + 1)].copy()   # [S, 16, R, D]
        for jl in range(JL):
            wt[JL * c + jl, jl] = 0.0
        wt = np.roll(wt, -JL * c, axis=0)
        wt = wt.reshape(G // 2, 2, IL, JL, R, D).transpose(0, 2, 4, 1, 3, 5)
        m["wtgt"] = np.ascontiguousarray(
            wt.reshape(G // 2, 128, 2, JL, D)
        ).astype(BF16_NP)
        in_maps.append(m)
    return in_maps


def run(inputs, trace=False):
    nc = _get_nc()
    in_maps = _prep_in_maps(inputs)
    res = bass_utils.run_bass_kernel_spmd(
        nc, in_maps, core_ids=list(range(NC)), trace=trace
    )
    out = np.stack(
        [
            np.asarray(res.results[c]["lg_out"], dtype=np.float32)
            .transpose(2, 1, 0, 3)
            .reshape(L, V)
            for c in range(NC)
        ],
        axis=0,
    )
    return out, res


def kernel(**inputs):
    out, _ = run(inputs, trace=False)
    return out
